# revision 11
# baseline (speedup 1.0000x reference)
"""2-hop GCN (gnn_message_passing) on 8 trn2 NeuronCores via Bass.

Algorithm (reference): h = A_hat^2 x;  out = log_softmax(h @ W + b)
  A_hat = D^-1/2 (A + I) D^-1/2  with D = in-degree+1 of (A + I).

Restructured for the device (per hop, per core; self-loops folded into the
drain as an elementwise term):
  S_t   = sum_{e: tgt=t} Z[src_e]        (edge segment-sum)
  out_t = scale_t * (S_t + dinv_t * Zself_t)   [+ bias on hop 2]

Device pattern per core ("scan hop"), all SBUF-resident:
  - Z table [128p, 6256, 4] fp16: partition 16g+ch holds features
    [4ch..4ch+4) of the g-th node-eighth (plus a zero row at 6250).
    Built from the allgathered Z by 8 plain DMAs (rearranged APs).
  - InstAPGather (Q7, 8 independent per-16-partition index streams: stream
    g = edges whose src is in node-eighth g) gathers per-edge messages
    into [128, SLABW, 4] fp16 slabs, edges sorted by target.
  - Gated tensor_tensor_scan (DVE, fp32 state): state = gate*state + msg
    with gate=0 at each target's first edge -> segmented prefix sums;
    value at a target's last edge is its segment total. In-place, 4
    stride-4 sub-streams (one per fp16 lane).
  - InstAPGather #2 extracts the per-target totals at compile-time-known
    end positions (zero-edge targets point at a zeroed pad slot).
  - PE combine: 4 accumulating matmuls per 128-target chunk with one-hot
    R_r matrices sum the 8 streams' partials and de-interleave features,
    producing node-major [128t, 40] in PSUM.
  - Drain: (psum + self-term) * dinv-scale (+ bias / log_softmax on hop 2).

Between hops: AllGather of the fp16 node shard (as before).
"""
import math

import numpy as np

import concourse.bacc as bacc
import concourse.bass as bass
import concourse.mybir as mybir
import concourse.tile as tile
from concourse.bass_utils import run_bass_kernel_spmd
from concourse.masks import make_identity

F32 = mybir.dt.float32
F16 = mybir.dt.float16
I16 = mybir.dt.int16

LAST_RESULTS = None  # test harness peeks at this after kernel(..., _trace=True)


class Cfg:
    def __init__(self, N=50000, F=100, C=40, CORES=8):
        self.N, self.F, self.C, self.CORES = N, F, C, CORES
        self.NPC = N // CORES          # targets per core
        self.NLOC = N // 8             # nodes per stream table (8 streams)
        self.ZROW = self.NLOC          # zero-row index in the table
        self.NE_T = self.NLOC + 16     # table elems (incl zero row + pad)
        self.NSLAB = 7                 # slabs per hop
        self.TRANGE = 896              # targets per slab (7 chunks of 128)
        self.NCHUNK = 49               # total 128-target chunks (49*128>=6250)
        self.EXT_W = self.TRANGE       # extraction idxs per slab
        self.NT_HEAD = math.ceil(self.NPC / 128)
        assert self.NSLAB * self.TRANGE >= self.NPC
        assert self.NCHUNK == self.NSLAB * (self.TRANGE // 128)


def _wrap16(flat):
    """idx position i -> [i%16, i//16] (per-group wrapped layout)."""
    n = flat.shape[0]
    assert n % 16 == 0
    return flat.reshape(n // 16, 16).T  # [16, n/16]


def preprocess(edge_index, cfg):
    N, NPC, NLOC, NS = cfg.N, cfg.NPC, cfg.NLOC, cfg.NSLAB
    TR = cfg.TRANGE
    row = np.asarray(edge_index[0], dtype=np.int64)
    col = np.asarray(edge_index[1], dtype=np.int64)

    deg = np.bincount(col, minlength=N).astype(np.float64) + 1.0
    dinv = 1.0 / np.sqrt(deg)

    cores = []
    counts = np.zeros((cfg.CORES, 8, NS), np.int64)
    for c in range(cfg.CORES):
        lo = c * NPC
        m = (col >= lo) & (col < lo + NPC)
        src, tgt = row[m], col[m] - lo
        g = src // NLOC
        sl = src % NLOC
        s = np.minimum(tgt // TR, NS - 1)
        order = np.lexsort((tgt, s, g))
        src, tgt, g, sl, s = (a[order] for a in (src, tgt, g, sl, s))
        cores.append((tgt, g, sl, s))
        for gi in range(8):
            for si in range(NS):
                counts[c, gi, si] = np.sum((g == gi) & (s == si))

    SLABW = int(counts.max()) + 1
    SLABW += (-SLABW) % 16
    cfg.SLABW = SLABW
    GW = NS * (SLABW // 16)       # gidx cols
    EW = NS * (cfg.EXT_W // 16)   # eidx cols
    sched = dict(SLABW=SLABW, GW=GW, EW=EW)

    per_core = []
    for c in range(cfg.CORES):
        tgt, g, sl, s = cores[c]
        gidx = np.full((8, NS, SLABW), cfg.ZROW, np.int64)
        gate = np.ones((8, NS, SLABW), np.float16)
        eidx = np.zeros((8, NS, cfg.EXT_W), np.int64)
        for gi in range(8):
            gm = g == gi
            for si in range(NS):
                mm = gm & (s == si)
                tt, ss = tgt[mm], sl[mm]
                cnt = tt.shape[0]
                gidx[gi, si, :cnt] = ss
                first = np.ones(cnt, bool)
                first[1:] = tt[1:] != tt[:-1]
                gate[gi, si, :cnt][first] = 0.0
                gate[gi, si, cnt] = 0.0  # sentinel pad -> scan value 0
                eidx[gi, si, :] = cnt    # default: sentinel (zero)
                last = np.ones(cnt, bool)
                last[:-1] = tt[1:] != tt[:-1]
                lp = np.nonzero(last)[0]
                lt = tt[lp] - si * TR    # local target in slab
                eidx[gi, si, lt] = lp
        # wrapped per-group layouts
        gw = np.zeros((128, GW), np.int16)
        ew = np.zeros((128, EW), np.int16)
        for gi in range(8):
            for si in range(NS):
                gw[16 * gi:16 * gi + 16,
                   si * (SLABW // 16):(si + 1) * (SLABW // 16)] = \
                    _wrap16(gidx[gi, si].astype(np.int16))
                ew[16 * gi:16 * gi + 16,
                   si * (cfg.EXT_W // 16):(si + 1) * (cfg.EXT_W // 16)] = \
                    _wrap16(eidx[gi, si].astype(np.int16))
        gate_t = gate.reshape(8, NS * SLABW).astype(np.float16)

        lo = c * NPC
        dl = dinv[lo:lo + NPC]
        d1A = np.zeros((128, cfg.NT_HEAD), np.float32)
        d2A = np.zeros((128, cfg.NT_HEAD), np.float32)
        for b in range(cfg.NT_HEAD):
            r = min(128, NPC - b * 128)
            d1A[:r, b] = dl[b * 128:b * 128 + r]
            d2A[:r, b] = dl[b * 128:b * 128 + r] ** 2
        per_core.append(dict(gidx=gw, eidx=ew, gate=gate_t, d1A=d1A, d2A=d2A))

    # combine one-hots: R[r, 16g+ch, 4ch+r] = 1
    R = np.zeros((4, 128, cfg.C), np.float16)
    for r in range(4):
        for gi in range(8):
            for ch in range(10):
                R[r, 16 * gi + ch, 4 * ch + r] = 1.0
    for pc in per_core:
        pc["R"] = R
    return sched, per_core


def build_program(cfg, sched):
    nc = bacc.Bacc("TRN2", target_bir_lowering=False, debug=False,
                   num_devices=cfg.CORES)
    N, F_, C, NPC = cfg.N, cfg.F, cfg.C, cfg.NPC
    NS, SLABW, TR, EXT_W = cfg.NSLAB, cfg.SLABW, cfg.TRANGE, cfg.EXT_W
    NB = cfg.NT_HEAD        # 49 chunks
    CPS = TR // 128         # chunks per slab (7)
    GW, EW = sched["GW"], sched["EW"]
    SW16, EW16 = SLABW // 16, EXT_W // 16

    xs = nc.dram_tensor("xs", [NPC, F_], F32, kind="ExternalInput")
    W = nc.dram_tensor("W", [F_, C], F32, kind="ExternalInput")
    bvec = nc.dram_tensor("bvec", [1, C], F32, kind="ExternalInput")
    d1A = nc.dram_tensor("d1A", [128, NB], F32, kind="ExternalInput")
    d2A = nc.dram_tensor("d2A", [128, NB], F32, kind="ExternalInput")
    Rm = nc.dram_tensor("Rm", [4, 128, C], F16, kind="ExternalInput")
    gidx = nc.dram_tensor("gidx", [128, GW], I16, kind="ExternalInput")
    eidx = nc.dram_tensor("eidx", [128, EW], I16, kind="ExternalInput")
    gate = nc.dram_tensor("gate", [8, NS * SLABW], F16, kind="ExternalInput")
    out = nc.dram_tensor("out", [NPC, C], F32, kind="ExternalOutput")

    cc1_in = nc.dram_tensor("cc1_in", [NPC, C], F16)
    cc2_in = nc.dram_tensor("cc2_in", [NPC, C], F16)
    z1c = nc.dram_tensor("z1c", [N, C], F16, addr_space="Shared")
    z2c = nc.dram_tensor("z2c", [N, C], F16, addr_space="Shared")

    ntail = NPC - (NB - 1) * 128  # valid rows in last chunk (106)

    with tile.TileContext(nc) as tc:
        with tc.tile_pool(name="const", bufs=1) as cpool, \
             tc.tile_pool(name="psum", bufs=8, space="PSUM") as psum_pool, \
             tc.tile_pool(name="sb", bufs=3) as sb, \
             tc.tile_pool(name="tab", bufs=1) as tabpool, \
             tc.tile_pool(name="msg", bufs=2) as msgpool, \
             tc.tile_pool(name="ext", bufs=2) as extpool, \
             tc.tile_pool(name="drain", bufs=1) as dpool:

            ident = cpool.tile([128, 128], F32)
            make_identity(nc, ident[:])
            W_sb = cpool.tile([F_, C], F32)
            nc.sync.dma_start(out=W_sb[:], in_=W[:, :])
            d1A_sb = cpool.tile([128, NB], F32)
            nc.sync.dma_start(out=d1A_sb[:], in_=d1A[:, :])
            d2A_sb = cpool.tile([128, NB], F32)
            nc.sync.dma_start(out=d2A_sb[:], in_=d2A[:, :])
            b_sb = cpool.tile([128, C], F32)
            nc.sync.dma_start(out=b_sb[:], in_=bvec[:, :].to_broadcast([128, C]))
            R_sb = cpool.tile([128, 4, C], F16)
            nc.sync.dma_start(out=R_sb[:],
                              in_=Rm[:, :, :].rearrange("r p c -> p r c"))
            gidx_sb = cpool.tile([128, GW], I16)
            nc.sync.dma_start(out=gidx_sb[:], in_=gidx[:, :])
            eidx_sb = cpool.tile([128, EW], I16)
            nc.sync.dma_start(out=eidx_sb[:], in_=eidx[:, :])
            gate_sb = cpool.tile([128, NS * SLABW], F16)
            for g in range(8):
                nc.sync.dma_start(
                    out=gate_sb[16 * g:16 * g + 16, :],
                    in_=gate[g:g + 1, :].to_broadcast([16, NS * SLABW]))

            T = tabpool.tile([128, cfg.NE_T, 4], F16)
            nc.vector.memset(T[:], 0.0)
            self1 = dpool.tile([128, NB, C], F16, tag="self1")
            nc.vector.memset(self1[:], 0.0)
            self2 = dpool.tile([128, NB, C], F16, tag="self2")
            nc.vector.memset(self2[:], 0.0)

            # ---- head: Z0 = dinv * (xs @ W), cast fp16 -> cc1_in ----
            for p in range(NB):
                rows = min(128, NPC - p * 128)
                xt = sb.tile([128, F_], F32, tag="xt")
                if rows < 128:
                    nc.vector.memset(xt[:], 0.0)
                nc.sync.dma_start(out=xt[:rows, :],
                                  in_=xs[p * 128:p * 128 + rows, :])
                bank = psum_pool.tile([128, 512], F32, tag="bank")
                nc.tensor.transpose(out=bank[0:F_, 0:128], in_=xt[:],
                                    identity=ident[:])
                xT = sb.tile([F_, 128], F32, tag="xT")
                nc.vector.tensor_copy(out=xT[:], in_=bank[0:F_, 0:128])
                bank2 = psum_pool.tile([128, 512], F32, tag="bank")
                nc.tensor.matmul(out=bank2[0:rows, 0:C], lhsT=xT[:, :rows],
                                 rhs=W_sb[:], start=True, stop=True)
                z0 = sb.tile([128, C], F16, tag="z0")
                nc.vector.tensor_scalar_mul(
                    z0[:rows, :], bank2[0:rows, 0:C], d1A_sb[:rows, p:p + 1])
                nc.sync.dma_start(out=cc1_in[p * 128:p * 128 + rows, :],
                                  in_=z0[:rows, :])

            def load_self(tile_, cc_in):
                nc.scalar.dma_start(
                    out=tile_[:, 0:NB - 1, :],
                    in_=cc_in[0:(NB - 1) * 128, :].rearrange(
                        "(b p) c -> p b c", p=128))
                nc.scalar.dma_start(
                    out=tile_[0:ntail, NB - 1, :],
                    in_=cc_in[(NB - 1) * 128:NPC, :])

            def allgather(cc_in, z_c):
                nc.gpsimd.collective_compute(
                    "AllGather", mybir.AluOpType.bypass,
                    replica_groups=[list(range(cfg.CORES))],
                    ins=[cc_in[:, :].opt()], outs=[z_c[:, :].opt()])

            load_self(self1, cc1_in)
            allgather(cc1_in, z1c)

            def emit_hop(z_c, self_t, drain, tag):
                # table: 8 DMAs, stream g <- node-eighth g
                for g in range(8):
                    nc.scalar.dma_start(
                        out=T[16 * g:16 * g + 10, 0:cfg.NLOC, :],
                        in_=z_c[g * cfg.NLOC:(g + 1) * cfg.NLOC, :].rearrange(
                            "n (c d) -> c n d", d=4))
                for s in range(NS):
                    msg = msgpool.tile([128, SLABW, 4], F16, tag="msg")
                    nc.gpsimd.ap_gather(
                        out_ap=msg[:], in_ap=T[:],
                        idxs_ap=gidx_sb[:, s * SW16:(s + 1) * SW16],
                        channels=128, num_elems=cfg.NE_T, d=4,
                        num_idxs=SLABW)
                    for r in range(4):
                        nc.vector.tensor_tensor_scan(
                            out=msg[:, :, r],
                            data0=gate_sb[:, s * SLABW:(s + 1) * SLABW],
                            data1=msg[:, :, r], initial=0.0,
                            op0=mybir.AluOpType.mult,
                            op1=mybir.AluOpType.add)
                    X = extpool.tile([128, EXT_W, 4], F16, tag="X")
                    nc.gpsimd.ap_gather(
                        out_ap=X[:], in_ap=msg[:],
                        idxs_ap=eidx_sb[:, s * EW16:(s + 1) * EW16],
                        channels=128, num_elems=SLABW, d=4,
                        num_idxs=EXT_W)
                    for k in range(CPS):
                        gb = s * CPS + k
                        if gb >= NB:
                            break
                        bank = psum_pool.tile([128, 512], F32, tag="bank",
                                              name=f"{tag}_b{gb}")
                        for r in range(4):
                            nc.tensor.matmul(
                                out=bank[0:128, 0:C],
                                lhsT=X[:, k * 128:(k + 1) * 128, r],
                                rhs=R_sb[:, r, :],
                                start=(r == 0), stop=(r == 3))
                        drain(gb, bank[0:128, 0:C])

            # ---- hop1 ----
            hop1_as = dpool.tile([128, NB, C], F16, tag="asm")

            def drain1(gb, psum_ap):
                nc.vector.tensor_tensor(
                    out=hop1_as[:, gb, :], in0=psum_ap,
                    in1=self1[:, gb, :], op=mybir.AluOpType.add)
                nc.vector.tensor_scalar_mul(
                    hop1_as[:, gb, :], hop1_as[:, gb, :],
                    d2A_sb[:, gb:gb + 1])

            emit_hop(z1c, self1, drain1, "h1")
            nc.sync.dma_start(
                out=cc2_in[0:(NB - 1) * 128, :].rearrange(
                    "(b p) c -> p b c", p=128),
                in_=hop1_as[:, 0:NB - 1, :])
            nc.sync.dma_start(out=cc2_in[(NB - 1) * 128:NPC, :],
                              in_=hop1_as[0:ntail, NB - 1, :])

            load_self(self2, cc2_in)
            allgather(cc2_in, z2c)

            # ---- hop2 ----
            hop2_as = dpool.tile([128, NB, C], F32, tag="asm2")

            def drain2(gb, psum_ap):
                nc.vector.tensor_tensor(
                    out=hop2_as[:, gb, :], in0=psum_ap,
                    in1=self2[:, gb, :], op=mybir.AluOpType.add)
                nc.vector.tensor_scalar_mul(
                    hop2_as[:, gb, :], hop2_as[:, gb, :],
                    d1A_sb[:, gb:gb + 1])
                nc.vector.tensor_add(
                    out=hop2_as[:, gb, :], in0=hop2_as[:, gb, :], in1=b_sb[:])

            emit_hop(z2c, self2, drain2, "h2")

            # ---- log_softmax over C (free axis) ----
            mx = dpool.tile([128, NB], F32, tag="mx")
            nc.vector.tensor_reduce(out=mx[:], in_=hop2_as[:],
                                    axis=mybir.AxisListType.X,
                                    op=mybir.AluOpType.max)
            tshift = dpool.tile([128, NB, C], F32, tag="tshift")
            nc.vector.tensor_tensor(
                out=tshift[:], in0=hop2_as[:],
                in1=mx[:].unsqueeze(2).to_broadcast([128, NB, C]),
                op=mybir.AluOpType.subtract)
            ex = dpool.tile([128, NB, C], F32, tag="ex")
            nc.scalar.activation(out=ex[:], in_=tshift[:],
                                 func=mybir.ActivationFunctionType.Exp)
            sm = dpool.tile([128, NB], F32, tag="sm")
            nc.vector.tensor_reduce(out=sm[:], in_=ex[:],
                                    axis=mybir.AxisListType.X,
                                    op=mybir.AluOpType.add)
            lsm = dpool.tile([128, NB], F32, tag="lsm")
            nc.scalar.activation(out=lsm[:], in_=sm[:],
                                 func=mybir.ActivationFunctionType.Ln)
            res = dpool.tile([128, NB, C], F32, tag="res")
            nc.vector.tensor_tensor(
                out=res[:], in0=tshift[:],
                in1=lsm[:].unsqueeze(2).to_broadcast([128, NB, C]),
                op=mybir.AluOpType.subtract)
            nc.sync.dma_start(
                out=out[0:(NB - 1) * 128, :].rearrange("(b p) c -> p b c",
                                                       p=128),
                in_=res[:, 0:NB - 1, :])
            nc.sync.dma_start(out=out[(NB - 1) * 128:NPC, :],
                              in_=res[0:ntail, NB - 1, :])
    nc.compile()
    return nc


def kernel(x, edge_index, W, b, _cfg=None, _trace=False, _sim=False):
    global LAST_RESULTS
    cfg = _cfg or Cfg()
    x = np.asarray(x, dtype=np.float32)
    W_ = np.asarray(W, dtype=np.float32)
    b_ = np.asarray(b, dtype=np.float32).reshape(1, cfg.C)
    sched, per_core = preprocess(np.asarray(edge_index), cfg)
    nc = build_program(cfg, sched)

    in_maps = []
    for c in range(cfg.CORES):
        pc = per_core[c]
        in_maps.append({
            "xs": x[c * cfg.NPC:(c + 1) * cfg.NPC, :],
            "W": W_, "bvec": b_,
            "d1A": pc["d1A"], "d2A": pc["d2A"], "Rm": pc["R"],
            "gidx": pc["gidx"], "eidx": pc["eidx"], "gate": pc["gate"],
        })

    if _sim:
        import concourse.bass_interp as bass_interp
        sim = bass_interp.MultiCoreSim(nc, cfg.CORES)
        for c in range(cfg.CORES):
            for k, v in in_maps[c].items():
                sim.cores[c].tensor(k)[:] = v
        sim.simulate()
        outs = [np.array(sim.cores[c].mem_tensor("out"))
                for c in range(cfg.CORES)]
        return np.concatenate(outs, axis=0)

    if _trace:
        import ntff_shim  # noqa: F401
    res = run_bass_kernel_spmd(nc, in_maps, core_ids=list(range(cfg.CORES)),
                               trace=_trace)
    LAST_RESULTS = res
    return np.concatenate([res.results[c]["out"] for c in range(cfg.CORES)],
                          axis=0)


# revision 13
# speedup vs baseline: 1.6385x; 1.6385x over previous
"""2-hop GCN (gnn_message_passing) on 8 trn2 NeuronCores via Bass.

Algorithm (reference): h = A_hat^2 x;  out = log_softmax(h @ W + b)
  A_hat = D^-1/2 (A + I) D^-1/2  with D = in-degree+1 of (A + I).

Restructured per hop, per core (self-loops folded into the drain):
  S_t   = sum_{e: tgt=t} Z[src_e]        (edge segment-sum)
  out_t = scale_t * (S_t + Zself_t)      [+ bias on hop 2]

Device pattern ("scan hop"):
  - Each core's outgoing shard is written in TABLE layout cc[10, 6250, 4]
    fp16 (partition c holds features [4c..4c+4) of each node); the
    AllGather concatenates shards so the per-stream gather tables load
    with 8 fully-contiguous DMAs (no strided table build).
  - InstAPGather (Q7; 8 independent per-16-partition index streams, one
    per source node-eighth) gathers per-edge messages into
    [128, SLABW, 4] fp16 slabs, edges sorted by target.
  - Gated tensor_tensor_scan (DVE, fp32 state; gate=0 at each target's
    first edge) turns the slabs into segmented prefix sums; a target's
    total sits at its last edge's position.
  - InstAPGather #2 extracts totals at compile-time end positions
    (2 slabs per call to amortize launch overhead; zero-edge targets
    point at a zeroed pad slot).
  - PE combine: 4 accumulating matmuls per 128-target chunk (one-hot R_r
    sums the 8 streams and de-interleaves features) -> node-major PSUM.
  - Drain: (psum + self) * dinv-scale; per-chunk DMAs immediately write
    the next hop's table-layout shard (overlapped with later slabs).
"""
import math

import numpy as np

import concourse.bacc as bacc
import concourse.bass as bass
import concourse.mybir as mybir
import concourse.tile as tile
from concourse.bass_utils import run_bass_kernel_spmd
from concourse.masks import make_identity

F32 = mybir.dt.float32
F16 = mybir.dt.float16
I16 = mybir.dt.int16

LAST_RESULTS = None  # test harness peeks at this after kernel(..., _trace=True)


class Cfg:
    def __init__(self, N=50000, F=100, C=40, CORES=8):
        self.N, self.F, self.C, self.CORES = N, F, C, CORES
        self.NPC = N // CORES          # targets per core
        self.NLOC = N // 8             # nodes per stream table (8 streams)
        self.ZROW = self.NLOC          # zero-row index in the table
        self.NE_T = self.NLOC + 16     # table elems (incl zero row + pad)
        self.NSLAB = 7                 # slabs per hop
        self.TRANGE = 896              # targets per slab (7 chunks of 128)
        self.NCHUNK = 49
        self.EXT_W = self.TRANGE
        self.NT_HEAD = math.ceil(self.NPC / 128)
        # superslabs: extraction batches of 2 slabs
        self.SS = [(0, 1), (2, 3), (4, 5), (6,)]


def _wrap16(flat):
    n = flat.shape[0]
    assert n % 16 == 0
    return flat.reshape(n // 16, 16).T  # [16, n/16]


def preprocess(edge_index, cfg):
    N, NPC, NLOC, NS = cfg.N, cfg.NPC, cfg.NLOC, cfg.NSLAB
    TR = cfg.TRANGE
    row = np.asarray(edge_index[0], dtype=np.int64)
    col = np.asarray(edge_index[1], dtype=np.int64)

    deg = np.bincount(col, minlength=N).astype(np.float64) + 1.0
    dinv = 1.0 / np.sqrt(deg)

    cores = []
    counts = np.zeros((cfg.CORES, 8, NS), np.int64)
    for c in range(cfg.CORES):
        lo = c * NPC
        m = (col >= lo) & (col < lo + NPC)
        src, tgt = row[m], col[m] - lo
        g = src // NLOC
        sl = src % NLOC
        s = np.minimum(tgt // TR, NS - 1)
        order = np.lexsort((tgt, s, g))
        src, tgt, g, sl, s = (a[order] for a in (src, tgt, g, sl, s))
        cores.append((tgt, g, sl, s))
        for gi in range(8):
            for si in range(NS):
                counts[c, gi, si] = np.sum((g == gi) & (s == si))

    SLABW = int(counts.max()) + 1
    SLABW += (-SLABW) % 16
    cfg.SLABW = SLABW
    GW = NS * (SLABW // 16)
    EWTOT = sum(len(ss) * cfg.EXT_W for ss in cfg.SS) // 16
    sched = dict(SLABW=SLABW, GW=GW, EWTOT=EWTOT)

    per_core = []
    for c in range(cfg.CORES):
        tgt, g, sl, s = cores[c]
        gidx = np.full((8, NS, SLABW), cfg.ZROW, np.int64)
        gate = np.ones((8, NS, SLABW), np.float16)
        eidx = np.zeros((8, NS, cfg.EXT_W), np.int64)
        for gi in range(8):
            gm = g == gi
            for si in range(NS):
                mm = gm & (s == si)
                tt, ss_ = tgt[mm], sl[mm]
                cnt = tt.shape[0]
                gidx[gi, si, :cnt] = ss_
                first = np.ones(cnt, bool)
                first[1:] = tt[1:] != tt[:-1]
                gate[gi, si, :cnt][first] = 0.0
                gate[gi, si, cnt] = 0.0  # sentinel pad -> scan value 0
                eidx[gi, si, :] = cnt    # default: sentinel (zero)
                last = np.ones(cnt, bool)
                last[:-1] = tt[1:] != tt[:-1]
                lp = np.nonzero(last)[0]
                lt = tt[lp] - si * TR
                eidx[gi, si, lt] = lp
        gw = np.zeros((128, GW), np.int16)
        ew = np.zeros((128, EWTOT), np.int16)
        for gi in range(8):
            for si in range(NS):
                gw[16 * gi:16 * gi + 16,
                   si * (SLABW // 16):(si + 1) * (SLABW // 16)] = \
                    _wrap16(gidx[gi, si].astype(np.int16))
            off = 0
            for ss in cfg.SS:
                ev = np.concatenate(
                    [eidx[gi, si] + (si - ss[0]) * SLABW for si in ss])
                w = cfg.EXT_W * len(ss) // 16
                ew[16 * gi:16 * gi + 16, off:off + w] = \
                    _wrap16(ev.astype(np.int16))
                off += w
        gate_t = gate.reshape(8, NS * SLABW).astype(np.float16)

        lo = c * NPC
        dl = dinv[lo:lo + NPC]
        d1A = np.zeros((128, cfg.NT_HEAD), np.float32)
        d2A = np.zeros((128, cfg.NT_HEAD), np.float32)
        for b in range(cfg.NT_HEAD):
            r = min(128, NPC - b * 128)
            d1A[:r, b] = dl[b * 128:b * 128 + r]
            d2A[:r, b] = dl[b * 128:b * 128 + r] ** 2
        per_core.append(dict(gidx=gw, eidx=ew, gate=gate_t, d1A=d1A, d2A=d2A))

    R = np.zeros((4, 128, cfg.C), np.float16)
    for r in range(4):
        for gi in range(8):
            for ch in range(10):
                R[r, 16 * gi + ch, 4 * ch + r] = 1.0
    for pc in per_core:
        pc["R"] = R
    return sched, per_core


def build_program(cfg, sched):
    nc = bacc.Bacc("TRN2", target_bir_lowering=False, debug=False,
                   num_devices=cfg.CORES)
    N, F_, C, NPC = cfg.N, cfg.F, cfg.C, cfg.NPC
    NS, SLABW, TR, EXT_W = cfg.NSLAB, cfg.SLABW, cfg.TRANGE, cfg.EXT_W
    NB = cfg.NT_HEAD
    CPS = TR // 128
    GW, EWTOT = sched["GW"], sched["EWTOT"]
    SW16 = SLABW // 16

    xs = nc.dram_tensor("xs", [NPC, F_], F32, kind="ExternalInput")
    W = nc.dram_tensor("W", [F_, C], F32, kind="ExternalInput")
    bvec = nc.dram_tensor("bvec", [1, C], F32, kind="ExternalInput")
    d1A = nc.dram_tensor("d1A", [128, NB], F32, kind="ExternalInput")
    d2A = nc.dram_tensor("d2A", [128, NB], F32, kind="ExternalInput")
    Rm = nc.dram_tensor("Rm", [4, 128, C], F16, kind="ExternalInput")
    gidx = nc.dram_tensor("gidx", [128, GW], I16, kind="ExternalInput")
    eidx = nc.dram_tensor("eidx", [128, EWTOT], I16, kind="ExternalInput")
    gate = nc.dram_tensor("gate", [8, NS * SLABW], F16, kind="ExternalInput")
    out = nc.dram_tensor("out", [NPC, C], F32, kind="ExternalOutput")

    cc1p = nc.dram_tensor("cc1p", [10, NPC, 4], F16)
    cc2p = nc.dram_tensor("cc2p", [10, NPC, 4], F16)
    z1a = nc.dram_tensor("z1a", [80, NPC, 4], F16, addr_space="Shared")
    z2a = nc.dram_tensor("z2a", [80, NPC, 4], F16, addr_space="Shared")

    ntail = NPC - (NB - 1) * 128  # 106

    with tile.TileContext(nc) as tc:
        with tc.tile_pool(name="const", bufs=1) as cpool, \
             tc.tile_pool(name="psum", bufs=8, space="PSUM") as psum_pool, \
             tc.tile_pool(name="sb", bufs=3) as sb, \
             tc.tile_pool(name="tab", bufs=1) as tabpool, \
             tc.tile_pool(name="msg", bufs=2) as msgpool, \
             tc.tile_pool(name="ext", bufs=2) as extpool, \
             tc.tile_pool(name="drain", bufs=1) as dpool:

            ident = cpool.tile([128, 128], F32)
            make_identity(nc, ident[:])
            W_sb = cpool.tile([F_, C], F32)
            nc.sync.dma_start(out=W_sb[:], in_=W[:, :])
            d1A_sb = cpool.tile([128, NB], F32)
            nc.sync.dma_start(out=d1A_sb[:], in_=d1A[:, :])
            d2A_sb = cpool.tile([128, NB], F32)
            nc.sync.dma_start(out=d2A_sb[:], in_=d2A[:, :])
            b_sb = cpool.tile([128, C], F32)
            nc.sync.dma_start(out=b_sb[:], in_=bvec[:, :].to_broadcast([128, C]))
            R_sb = cpool.tile([128, 4, C], F16)
            nc.sync.dma_start(out=R_sb[:],
                              in_=Rm[:, :, :].rearrange("r p c -> p r c"))
            gidx_sb = cpool.tile([128, GW], I16)
            nc.sync.dma_start(out=gidx_sb[:], in_=gidx[:, :])
            eidx_sb = cpool.tile([128, EWTOT], I16)
            nc.sync.dma_start(out=eidx_sb[:], in_=eidx[:, :])
            gate_sb = cpool.tile([128, NS * SLABW], F16)
            for g in range(8):
                nc.sync.dma_start(
                    out=gate_sb[16 * g:16 * g + 16, :],
                    in_=gate[g:g + 1, :].to_broadcast([16, NS * SLABW]))

            T = tabpool.tile([128, cfg.NE_T, 4], F16)
            nc.vector.memset(T[:], 0.0)
            self1 = dpool.tile([128, NB, C], F16, tag="self1")
            nc.vector.memset(self1[:], 0.0)
            hop1_as = dpool.tile([128, NB, C], F16, tag="asm")
            nc.vector.memset(hop1_as[:], 0.0)

            def cc_write(cc_p, src_tile, b):
                """Write chunk b of a node-major [128, NB, C] tile into the
                table-layout shard cc_p[10, NPC, 4]."""
                eng = nc.sync if b % 2 == 0 else nc.scalar
                rows = min(128, NPC - b * 128)
                eng.dma_start(
                    out=cc_p[:, b * 128:b * 128 + rows, :].rearrange(
                        "c p d -> p c d"),
                    in_=src_tile[0:rows, b, :].rearrange(
                        "p (c d) -> p c d", d=4))

            # ---- head: Z0 = dinv * (xs @ W), fp16 -> self1 + cc1p ----
            for p in range(NB):
                rows = min(128, NPC - p * 128)
                xt = sb.tile([128, F_], F32, tag="xt")
                if rows < 128:
                    nc.vector.memset(xt[:], 0.0)
                nc.sync.dma_start(out=xt[:rows, :],
                                  in_=xs[p * 128:p * 128 + rows, :])
                bank = psum_pool.tile([128, 512], F32, tag="bank")
                nc.tensor.transpose(out=bank[0:F_, 0:128], in_=xt[:],
                                    identity=ident[:])
                xT = sb.tile([F_, 128], F32, tag="xT")
                nc.vector.tensor_copy(out=xT[:], in_=bank[0:F_, 0:128])
                bank2 = psum_pool.tile([128, 512], F32, tag="bank")
                nc.tensor.matmul(out=bank2[0:rows, 0:C], lhsT=xT[:, :rows],
                                 rhs=W_sb[:], start=True, stop=True)
                nc.vector.tensor_scalar_mul(
                    self1[:rows, p, :], bank2[0:rows, 0:C],
                    d1A_sb[:rows, p:p + 1])
                cc_write(cc1p, self1, p)

            def allgather(cc_in, z_a):
                nc.gpsimd.collective_compute(
                    "AllGather", mybir.AluOpType.bypass,
                    replica_groups=[list(range(cfg.CORES))],
                    ins=[cc_in[:, :, :].opt()], outs=[z_a[:, :, :].opt()])

            allgather(cc1p, z1a)

            def emit_hop(z_a, self_t, drain, tag):
                for g in range(8):
                    nc.scalar.dma_start(
                        out=T[16 * g:16 * g + 10, 0:cfg.NLOC, :],
                        in_=z_a[10 * g:10 * g + 10, :, :])
                eoff = 0
                for ss in cfg.SS:
                    nss = len(ss)
                    msg = msgpool.tile([128, 2, SLABW, 4], F16, tag="msg")
                    for j, s in enumerate(ss):
                        nc.gpsimd.ap_gather(
                            out_ap=msg[:, j, :, :], in_ap=T[:],
                            idxs_ap=gidx_sb[:, s * SW16:(s + 1) * SW16],
                            channels=128, num_elems=cfg.NE_T, d=4,
                            num_idxs=SLABW)
                        for r in range(4):
                            nc.vector.tensor_tensor_scan(
                                out=msg[:, j, :, r],
                                data0=gate_sb[:, s * SLABW:(s + 1) * SLABW],
                                data1=msg[:, j, :, r], initial=0.0,
                                op0=mybir.AluOpType.mult,
                                op1=mybir.AluOpType.add)
                    ew = nss * EXT_W
                    X = extpool.tile([128, 2 * EXT_W, 4], F16, tag="X")
                    nc.gpsimd.ap_gather(
                        out_ap=X[:, 0:ew, :],
                        in_ap=msg[:, 0:nss, :, :],
                        idxs_ap=eidx_sb[:, eoff:eoff + ew // 16],
                        channels=128, num_elems=nss * SLABW, d=4,
                        num_idxs=ew)
                    eoff += ew // 16
                    for k in range(nss * CPS):
                        gb = ss[0] * CPS + k
                        bank = psum_pool.tile([128, 512], F32, tag="bank",
                                              name=f"{tag}_b{gb}")
                        for r in range(4):
                            nc.tensor.matmul(
                                out=bank[0:128, 0:C],
                                lhsT=X[:, k * 128:(k + 1) * 128, r],
                                rhs=R_sb[:, r, :],
                                start=(r == 0), stop=(r == 3))
                        drain(gb, bank[0:128, 0:C])

            # ---- hop1: Z2 = d2 * (psum + Z0self) -> hop1_as + cc2p ----
            def drain1(gb, psum_ap):
                nc.vector.tensor_tensor(
                    out=hop1_as[:, gb, :], in0=psum_ap,
                    in1=self1[:, gb, :], op=mybir.AluOpType.add)
                nc.vector.tensor_scalar_mul(
                    hop1_as[:, gb, :], hop1_as[:, gb, :],
                    d2A_sb[:, gb:gb + 1])
                cc_write(cc2p, hop1_as, gb)

            emit_hop(z1a, self1, drain1, "h1")
            allgather(cc2p, z2a)

            # ---- hop2: logits = d1 * (psum + Z2self) + b ----
            hop2_as = dpool.tile([128, NB, C], F32, tag="asm2")

            def drain2(gb, psum_ap):
                nc.vector.tensor_tensor(
                    out=hop2_as[:, gb, :], in0=psum_ap,
                    in1=hop1_as[:, gb, :], op=mybir.AluOpType.add)
                nc.vector.tensor_scalar_mul(
                    hop2_as[:, gb, :], hop2_as[:, gb, :],
                    d1A_sb[:, gb:gb + 1])
                nc.vector.tensor_add(
                    out=hop2_as[:, gb, :], in0=hop2_as[:, gb, :], in1=b_sb[:])

            emit_hop(z2a, hop1_as, drain2, "h2")

            # ---- log_softmax over C (free axis) ----
            mx = dpool.tile([128, NB], F32, tag="mx")
            nc.vector.tensor_reduce(out=mx[:], in_=hop2_as[:],
                                    axis=mybir.AxisListType.X,
                                    op=mybir.AluOpType.max)
            nc.vector.tensor_tensor(
                out=hop2_as[:], in0=hop2_as[:],
                in1=mx[:].unsqueeze(2).to_broadcast([128, NB, C]),
                op=mybir.AluOpType.subtract)
            ex = dpool.tile([128, NB, C], F32, tag="ex")
            nc.scalar.activation(out=ex[:], in_=hop2_as[:],
                                 func=mybir.ActivationFunctionType.Exp)
            sm = dpool.tile([128, NB], F32, tag="sm")
            nc.vector.tensor_reduce(out=sm[:], in_=ex[:],
                                    axis=mybir.AxisListType.X,
                                    op=mybir.AluOpType.add)
            lsm = dpool.tile([128, NB], F32, tag="lsm")
            nc.scalar.activation(out=lsm[:], in_=sm[:],
                                 func=mybir.ActivationFunctionType.Ln)
            nc.vector.tensor_tensor(
                out=ex[:], in0=hop2_as[:],
                in1=lsm[:].unsqueeze(2).to_broadcast([128, NB, C]),
                op=mybir.AluOpType.subtract)
            nc.sync.dma_start(
                out=out[0:(NB - 1) * 128, :].rearrange("(b p) c -> p b c",
                                                       p=128),
                in_=ex[:, 0:NB - 1, :])
            nc.sync.dma_start(out=out[(NB - 1) * 128:NPC, :],
                              in_=ex[0:ntail, NB - 1, :])
    nc.compile()
    return nc


def kernel(x, edge_index, W, b, _cfg=None, _trace=False, _sim=False):
    global LAST_RESULTS
    cfg = _cfg or Cfg()
    x = np.asarray(x, dtype=np.float32)
    W_ = np.asarray(W, dtype=np.float32)
    b_ = np.asarray(b, dtype=np.float32).reshape(1, cfg.C)
    sched, per_core = preprocess(np.asarray(edge_index), cfg)
    nc = build_program(cfg, sched)

    in_maps = []
    for c in range(cfg.CORES):
        pc = per_core[c]
        in_maps.append({
            "xs": x[c * cfg.NPC:(c + 1) * cfg.NPC, :],
            "W": W_, "bvec": b_,
            "d1A": pc["d1A"], "d2A": pc["d2A"], "Rm": pc["R"],
            "gidx": pc["gidx"], "eidx": pc["eidx"], "gate": pc["gate"],
        })

    if _sim:
        import concourse.bass_interp as bass_interp
        sim = bass_interp.MultiCoreSim(nc, cfg.CORES)
        for c in range(cfg.CORES):
            for k, v in in_maps[c].items():
                sim.cores[c].tensor(k)[:] = v
        sim.simulate()
        outs = [np.array(sim.cores[c].mem_tensor("out"))
                for c in range(cfg.CORES)]
        return np.concatenate(outs, axis=0)

    if _trace:
        import ntff_shim  # noqa: F401
    res = run_bass_kernel_spmd(nc, in_maps, core_ids=list(range(cfg.CORES)),
                               trace=_trace)
    LAST_RESULTS = res
    return np.concatenate([res.results[c]["out"] for c in range(cfg.CORES)],
                          axis=0)
